# revision 11
# baseline (speedup 1.0000x reference)
"""Trainium2 Bass kernel for nn_MemoryTimeUnit.

Math: the reference keeps only Zp[:, :P] and averages over V. By linearity the
computation collapses to (per batch):
  out = proj(feat) + bias table, with
  y_fwd[t,d]  = causal 64-tap conv of memory with kf          (DFT-128 on device)
  y_bwd[t,d]  = anticausal conv with kb + Re{g_b lam_b^{P-t} S[d]}
  S[d] = sum_{j<T} lam_b^j * mean_v ts[b,j,v,d]
The backward DFT of the flipped memory equals conj(fwd DFT) with phases that
cancel exactly in the inverse transform, so one DFT (Zr,Zi) serves both paths.
Sign/swap variants of the K-hat tables are precomputed on host so all four
pointwise complex products read the same [zr|zi] operand (no swapped copy),
and fwd/bwd share the C/Sm inverse tables.

|lam_b| < 1 decays exponentially in j, so S is truncated adaptively at the
smallest T in {64, 128, 256, ...} whose predicted output error 0.04*max|lam|^T
is under 4e-3 (computed from bwd_nu at runtime; measured truncation error at
T=64 on randn-scale params is ~3e-3 against the 2e-2 gate).

Sharding: one batch per core (8 cores). Tables host-precomputed from the
per-channel params (O(D^2), data-independent) and replicated. Inputs
host-cast to fp16. For T=64 the ts slice is shipped as [128, 1024] (same
bytes) so the v-reduction uses all 128 DVE lanes; the lam^j weight table is
row-duplicated to match. Engine split: 2-src elementwise on DVE only (GpSimd
shares SBUF ports with DVE), casts + per-partition scales on ACT, matmuls on
PE; three DMA streams (sync/scalar HWDGE + gpsimd SWDGE), all contiguous.
"""

import numpy as np

B, P, V, L_P, D = 8, 64, 8, 1024, 256
N = 128

_CACHE = {}
LAST_RESULTS = None


def _pick_T(bwd_nu):
    max_abs_lam = float(np.exp(-np.exp(np.asarray(bwd_nu, np.float64))).max())
    for T in [64, 128, 256, 384, 512, 640, 768, 896, 1024]:
        if 0.04 * max_abs_lam ** T <= 4e-3 or T == 1024:
            return T
    return 1024


def _make_tables(T, fwd_nu, fwd_theta, fwd_gr, fwd_gi, bwd_nu, bwd_theta,
                 bwd_gr, bwd_gi, proj_W, proj_b, prefix_emb, signal_emb):
    f64 = np.float64
    f16 = np.float16
    lam_f = np.exp(-np.exp(fwd_nu.astype(f64)) + 1j * fwd_theta.astype(f64))
    lam_b = np.exp(-np.exp(bwd_nu.astype(f64)) + 1j * bwd_theta.astype(f64))
    g_f = fwd_gr.astype(f64) + 1j * fwd_gi.astype(f64)
    g_b = bwd_gr.astype(f64) + 1j * bwd_gi.astype(f64)
    proj_Wd = proj_W.astype(f64)

    tau = np.arange(P)
    kf = np.real(g_f[None, :] * lam_f[None, :] ** tau[:, None])   # [64, D]
    kb = np.real(g_b[None, :] * lam_b[None, :] ** tau[:, None])
    Kf = np.fft.fft(kf, n=N, axis=0)                              # [128, D]
    Kb = np.fft.fft(kb, n=N, axis=0)
    Kfr, Kfi = np.real(Kf), np.imag(Kf)
    Kbr, Kbin = np.real(Kb), -np.imag(Kb)

    s = np.arange(N)
    f = np.arange(N)
    ang = 2 * np.pi * np.outer(s, f) / N
    FrT = np.cos(ang)                                             # [s, f]
    FiT = -np.sin(ang)
    t64 = np.arange(P)
    angi = 2 * np.pi * np.outer(f, t64) / N                       # [f, t]
    C = np.cos(angi) / N
    Sm = -np.sin(angi) / N

    jj = np.arange(T)
    lamj = lam_b[None, :] ** jj[:, None]                          # [T, D]
    Wr = np.real(lamj) / V
    Wi = np.imag(lamj) / V

    A = g_b[None, :] * lam_b[None, :] ** (P - t64)[:, None]       # [t, d]
    ArT = np.real(A).T                                            # [d, t]
    AinT = -np.imag(A).T

    cumkf = np.cumsum(kf, axis=0)
    cumkb = np.cumsum(kb, axis=0)
    pe = prefix_emb.reshape(-1).astype(f64)
    se = signal_emb.reshape(-1).astype(f64)
    y_pe_f = pe[None, :] * cumkf
    y_pe_b = pe[None, :] * cumkb[::-1, :]
    geo = np.sum(lam_b[None, :] ** np.arange(L_P)[:, None], axis=0)
    y_se_b = np.real(A * geo[None, :]) * se[None, :]
    Bfeat = np.concatenate([y_pe_f, y_pe_b + y_se_b], axis=1)     # [64, 2D]
    BT = (proj_b.astype(f64)[None, :] + Bfeat @ proj_Wd.T)        # [64, 256]

    # tblA tail [128, 256]: FrT | FiT  (joined to [x|mem] per batch)
    tblA_tail = np.concatenate([FrT, FiT], axis=1).astype(f16)
    # tblB: K1=[Kfr|Kfi] K2=[Kfi|Kfr] K3=[Kbr|-Kbi] K4=[-Kbi|Kbr] | C Sm |
    #       Ar_h0 Ar_h1 Ain_h0 Ain_h1 | W
    blocks = [Kfr, Kfi, Kfi, Kfr, Kbr, Kbin, Kbin, Kbr, C, Sm,
              ArT[:128], ArT[128:], AinT[:128], AinT[128:]]
    if T == 64:
        blocks.append(np.repeat(Wr, 2, axis=0))                   # [128, 256]
        blocks.append(np.repeat(Wi, 2, axis=0))
        n_wch = 1
    else:
        n_wch = T // 128
        for ch in range(n_wch):
            blocks.append(Wr[128 * ch:128 * (ch + 1)])
            blocks.append(Wi[128 * ch:128 * (ch + 1)])
    tblB = np.concatenate(blocks, axis=1).astype(f16)
    # tblC [128, 1280]: WP (4x[128,256]) | BT pad
    WpT = np.ascontiguousarray(proj_Wd.T)                         # [2D, 256]
    btpad = np.zeros((128, 256))
    btpad[:P] = BT
    tblC = np.concatenate([WpT[0:128], WpT[128:256], WpT[256:384],
                           WpT[384:512], btpad], axis=1).astype(f16)
    return {"tblA_tail": tblA_tail, "tblB": tblB, "tblC": tblC}


def _build_bass(T):
    import concourse.bacc as bacc
    import concourse.mybir as mybir
    from concourse.tile import TileContext

    dt = mybir.dt.float32
    dth = mybir.dt.float16
    nc = bacc.Bacc("TRN2", num_swdge_queues=1)

    n_ch = max(1, T // 128)
    WCOL = 2432                # K (2048) + C/Sm (128) + AT (256)
    if T == 64:
        tsx = nc.dram_tensor("tsx", (128, 1536), dth, kind="ExternalInput")
    else:
        tsx = nc.dram_tensor("tsx", (T, V * D), dth, kind="ExternalInput")
        tblAd = nc.dram_tensor("tblA", (N, 512), dth, kind="ExternalInput")
    tblBd = nc.dram_tensor("tblB", (N, WCOL + 512 * n_ch), dth,
                           kind="ExternalInput")
    tblCd = nc.dram_tensor("tblC", (N, 1280), dth, kind="ExternalInput")
    outd = nc.dram_tensor("out", (P, D), dt, kind="ExternalOutput")

    with TileContext(nc) as tc:
        with (
            tc.tile_pool(name="xin", bufs=min(n_ch, 2)) as xin_pool,
            tc.tile_pool(name="work", bufs=4) as work_pool,
            tc.tile_pool(name="const", bufs=1) as const_pool,
            tc.tile_pool(name="psz", bufs=1, space="PSUM") as psz_pool,
            tc.tile_pool(name="psf", bufs=1, space="PSUM") as psf_pool,
            tc.tile_pool(name="pst", bufs=1, space="PSUM") as pst_pool,
            tc.tile_pool(name="psp", bufs=1, space="PSUM") as psp_pool,
        ):
            # ---- input DMAs on three parallel rings ----
            tblB = const_pool.tile([N, WCOL + 512 * n_ch], dth)
            nc.scalar.dma_start(out=tblB[:], in_=tblBd[:])
            xs = []
            if T == 64:
                xa = xin_pool.tile([128, 1536], dth, tag="x")
                nc.sync.dma_start(out=xa[:], in_=tsx[:])
                xs.append(xa[:, 0:1024])
                tblA = xa[:, 1024:1536]
            else:
                tblA = const_pool.tile([N, 512], dth)
                nc.sync.dma_start(out=tblA[:], in_=tblAd[:])
                for ch in range(n_ch):
                    x = xin_pool.tile([128, 2048], dth, tag="x")
                    nc.sync.dma_start(out=x[:], in_=tsx[128 * ch:128 * ch + 128, :])
                    xs.append(x)
            tblC = const_pool.tile([N, 1280], dth)
            nc.gpsimd.dma_start(out=tblC[:], in_=tblCd[:])

            ones = const_pool.tile([128, 1], dth)
            nc.gpsimd.memset(ones[:], 1.0)

            mem_t = tblA[:, 0:256]
            FrT_t = tblA[:, 256:384]
            FiT_t = tblA[:, 384:512]
            Ct = tblB[:, 2048:2112]
            Smt = tblB[:, 2112:2176]
            ATt = tblB[:, 2176:2432]

            # ---- memory path: one DFT serves fwd+bwd ----
            zpsum = psz_pool.tile([N, 512], dt)
            nc.tensor.matmul(zpsum[:, 0:256], FrT_t, mem_t, start=True, stop=True)
            nc.tensor.matmul(zpsum[:, 256:512], FiT_t, mem_t, start=True, stop=True)
            zs = const_pool.tile([N, 512], dth)     # [zr|zi]
            nc.scalar.copy(out=zs[:], in_=zpsum[:])

            # ---- ts path: v-reduction + lam^j weighting ----
            st_psum = pst_pool.tile([128, 4], dt)
            if T == 64:
                x = xs[0]
                cc = work_pool.tile([128, 512], dth, tag="cc")
                nc.vector.tensor_add(out=cc[:], in0=x[:, 0:512], in1=x[:, 512:1024])
                a2 = work_pool.tile([128, 256], dth, tag="a2")
                nc.vector.tensor_add(out=a2[:], in0=cc[:, 0:256], in1=cc[:, 256:512])
                p = work_pool.tile([128, 512], dth, tag="p")
                nc.vector.tensor_mul(out=p[:, 0:256], in0=a2[:],
                                     in1=tblB[:, WCOL:WCOL + 256])
                nc.vector.tensor_mul(out=p[:, 256:512], in0=a2[:],
                                     in1=tblB[:, WCOL + 256:WCOL + 512])
                for g in range(4):
                    nc.tensor.matmul(st_psum[:, g:g + 1],
                                     p[:, 128 * g:128 * (g + 1)], ones[:],
                                     start=True, stop=True)
            else:
                for ch, x in enumerate(xs):
                    bb = work_pool.tile([128, 1024], dth, tag="bb")
                    nc.vector.tensor_add(out=bb[:], in0=x[:, 0:1024],
                                         in1=x[:, 1024:2048])
                    cc = work_pool.tile([128, 512], dth, tag="cc")
                    nc.vector.tensor_add(out=cc[:], in0=bb[:, 0:512],
                                         in1=bb[:, 512:1024])
                    a1 = work_pool.tile([128, 256], dth, tag="a2")
                    nc.vector.tensor_add(out=a1[:], in0=cc[:, 0:256],
                                         in1=cc[:, 256:512])
                    p = work_pool.tile([128, 512], dth, tag="p")
                    wof = WCOL + 512 * ch
                    nc.vector.tensor_mul(out=p[:, 0:256], in0=a1[:],
                                         in1=tblB[:, wof:wof + 256])
                    nc.vector.tensor_mul(out=p[:, 256:512], in0=a1[:],
                                         in1=tblB[:, wof + 256:wof + 512])
                    for g in range(4):
                        nc.tensor.matmul(st_psum[:, g:g + 1],
                                         p[:, 128 * g:128 * (g + 1)], ones[:],
                                         start=(ch == 0), stop=(ch == n_ch - 1))

            # ---- pointwise complex multiplies, 512-wide, all on [zr|zi] ----
            P1 = work_pool.tile([128, 512], dth, tag="P1")
            P2 = work_pool.tile([128, 512], dth, tag="P2")
            P3 = work_pool.tile([128, 512], dth, tag="P3")
            P4 = work_pool.tile([128, 512], dth, tag="P4")
            uf = const_pool.tile([128, 512], dth)    # [ufr|ufi]
            ub = const_pool.tile([128, 512], dth)    # [ubr|-ubi]
            nc.vector.tensor_mul(out=P1[:], in0=zs[:], in1=tblB[:, 0:512])
            nc.vector.tensor_mul(out=P2[:], in0=zs[:], in1=tblB[:, 512:1024])
            nc.vector.tensor_sub(out=uf[:, 0:256], in0=P1[:, 0:256], in1=P1[:, 256:512])
            nc.vector.tensor_add(out=uf[:, 256:512], in0=P2[:, 0:256], in1=P2[:, 256:512])
            nc.vector.tensor_mul(out=P3[:], in0=zs[:], in1=tblB[:, 1024:1536])
            nc.vector.tensor_mul(out=P4[:], in0=zs[:], in1=tblB[:, 1536:2048])
            nc.vector.tensor_sub(out=ub[:, 0:256], in0=P3[:, 0:256], in1=P3[:, 256:512])
            nc.vector.tensor_add(out=ub[:, 256:512], in0=P4[:, 0:256], in1=P4[:, 256:512])

            # ---- inverse transform into [d-block, t] feature blocks ----
            ffp = psf_pool.tile([128, 128], dt, tag="ffp")
            fbp = psf_pool.tile([128, 128], dt, tag="fbp")
            for h in range(2):
                nc.tensor.matmul(ffp[:, 64 * h:64 * h + 64],
                                 uf[:, 128 * h:128 * h + 128], Ct,
                                 start=True, stop=False)
                nc.tensor.matmul(ffp[:, 64 * h:64 * h + 64],
                                 uf[:, 256 + 128 * h:256 + 128 * h + 128], Smt,
                                 start=False, stop=True)
            for h in range(2):
                nc.tensor.matmul(fbp[:, 64 * h:64 * h + 64],
                                 ub[:, 128 * h:128 * h + 128], Ct,
                                 start=True, stop=False)
                nc.tensor.matmul(fbp[:, 64 * h:64 * h + 64],
                                 ub[:, 256 + 128 * h:256 + 128 * h + 128], Smt,
                                 start=False, stop=True)

            # ---- S-term (per-partition scales on ACT) + merge ----
            st_sb = const_pool.tile([128, 4], dt)
            nc.scalar.copy(out=st_sb[:], in_=st_psum[:])
            feat = const_pool.tile([128, 256], dth)
            uab = work_pool.tile([128, 128], dth, tag="uab")
            uaib = work_pool.tile([128, 128], dth, tag="uaib")
            for h in range(2):
                nc.scalar.mul(uab[:, 64 * h:64 * h + 64],
                              ATt[:, 64 * h:64 * h + 64], st_sb[:, h:h + 1])
                nc.scalar.mul(uaib[:, 64 * h:64 * h + 64],
                              ATt[:, 128 + 64 * h:192 + 64 * h],
                              st_sb[:, 2 + h:3 + h])
            fb1 = work_pool.tile([128, 128], dth, tag="fb1")
            nc.vector.tensor_add(out=fb1[:], in0=fbp[:], in1=uab[:])
            nc.vector.tensor_add(out=feat[:, 128:256], in0=fb1[:], in1=uaib[:])
            nc.scalar.copy(out=feat[:, 0:128], in_=ffp[:])

            # ---- projection + bias + store ----
            pj = psp_pool.tile([P, D], dt)
            for g in range(4):
                nc.tensor.matmul(pj[:], feat[:, 64 * g:64 * (g + 1)],
                                 tblC[:, 256 * g:256 * (g + 1)],
                                 start=(g == 0), stop=(g == 3))
            out_sb = const_pool.tile([P, D], dt)
            nc.vector.tensor_add(out=out_sb[:], in0=pj[:], in1=tblC[0:P, 1024:1280])
            nc.sync.dma_start(out=outd[:], in_=out_sb[:])

    nc.compile()
    return nc


def _ensure_axon_hooks_shim():
    """bass_utils imports antenv.axon_hooks when tracing; some images lack it."""
    import sys, types
    try:
        import antenv  # noqa: F401
    except ImportError:
        return
    if "antenv.axon_hooks" in sys.modules:
        return
    try:
        from antenv import axon_hooks  # noqa: F401
        return
    except ImportError:
        pass
    hooks = types.ModuleType("antenv.axon_hooks")
    hooks._hook = None
    def _set(h):
        hooks._hook = h
    def _get():
        return hooks._hook
    hooks.set_axon_ntff_profile_hook = _set
    hooks.get_axon_ntff_profile_hook = _get
    sys.modules["antenv.axon_hooks"] = hooks


def kernel(**inputs):
    global LAST_RESULTS
    import os
    from concourse.bass_utils import run_bass_kernel_spmd
    _ensure_axon_hooks_shim()

    f16 = np.float16
    T = _pick_T(inputs["bwd_nu"])

    if T not in _CACHE:
        _CACHE[T] = _build_bass(T)
    nc = _CACHE[T]

    pkeys = ["fwd_nu", "fwd_theta", "fwd_gr", "fwd_gi", "bwd_nu", "bwd_theta",
             "bwd_gr", "bwd_gi", "proj_W", "proj_b", "prefix_emb", "signal_emb"]
    tables = _make_tables(T, **{k: np.asarray(inputs[k]) for k in pkeys})
    tblA_tail = tables.pop("tblA_tail")

    memory = np.asarray(inputs["memory"], np.float32)
    ts_embeds = np.asarray(inputs["ts_embeds"], np.float32)
    in_maps = []
    if T == 64:
        ts16 = ts_embeds[:, :T].reshape(B, 128, 1024).astype(f16)
        for b in range(B):
            memp = np.zeros((N, D), f16)
            memp[:P] = memory[b]
            tsxA = np.concatenate([ts16[b], memp, tblA_tail], axis=1)
            m = {"tsx": tsxA}
            m.update(tables)
            in_maps.append(m)
    else:
        ts16 = ts_embeds[:, :T].reshape(B, T, V * D).astype(f16)
        for b in range(B):
            memp = np.zeros((N, D), f16)
            memp[:P] = memory[b]
            tblA = np.concatenate([memp, tblA_tail], axis=1)
            m = {"tsx": ts16[b], "tblA": tblA}
            m.update(tables)
            in_maps.append(m)

    trace = os.environ.get("BASS_KERNEL_TRACE", "0") == "1"
    res = run_bass_kernel_spmd(nc, in_maps, core_ids=list(range(B)), trace=trace)
    LAST_RESULTS = res
    return np.stack([res.results[b]["out"] for b in range(B)], axis=0)


# revision 12
# speedup vs baseline: 1.0329x; 1.0329x over previous
"""Trainium2 Bass kernel for nn_MemoryTimeUnit.

Math: the reference keeps only Zp[:, :P] and averages over V. By linearity the
computation collapses to (per batch):
  out = proj(feat) + bias table, with
  y_fwd[t,d]  = causal 64-tap conv of memory with kf          (DFT-128 on device)
  y_bwd[t,d]  = anticausal conv with kb + Re{g_b lam_b^{P-t} S[d]}
  S[d] = sum_{j<T} lam_b^j * mean_v ts[b,j,v,d]
The backward DFT of the flipped memory equals conj(fwd DFT) with phases that
cancel exactly in the inverse transform, so one DFT (Zr,Zi) serves both paths.
Sign/swap variants of the K-hat tables are precomputed on host so all four
pointwise complex products read the same [zr|zi] operand (no swapped copy),
and fwd/bwd share the C/Sm inverse tables.

|lam_b| < 1 decays exponentially in j, so S is truncated adaptively at the
smallest T in {64, 128, 256, ...} whose predicted output error 0.04*max|lam|^T
is under 4e-3 (computed from bwd_nu at runtime; measured truncation error at
T=64 on randn-scale params is ~3e-3 against the 2e-2 gate).

Sharding: one batch per core (8 cores). Tables host-precomputed from the
per-channel params (O(D^2), data-independent) and replicated. Inputs
host-cast to fp16. For T=64 the ts slice is shipped as [128, 1024] (same
bytes) so the v-reduction uses all 128 DVE lanes; the lam^j weight table is
row-duplicated to match. Engine split: 2-src elementwise on DVE only (GpSimd
shares SBUF ports with DVE), casts + per-partition scales on ACT, matmuls on
PE; three DMA streams (sync/scalar HWDGE + gpsimd SWDGE), all contiguous.
"""

import numpy as np

B, P, V, L_P, D = 8, 64, 8, 1024, 256
N = 128

_CACHE = {}
LAST_RESULTS = None


def _pick_T(bwd_nu):
    max_abs_lam = float(np.exp(-np.exp(np.asarray(bwd_nu, np.float64))).max())
    for T in [64, 128, 256, 384, 512, 640, 768, 896, 1024]:
        if 0.04 * max_abs_lam ** T <= 4e-3 or T == 1024:
            return T
    return 1024


def _make_tables(T, fwd_nu, fwd_theta, fwd_gr, fwd_gi, bwd_nu, bwd_theta,
                 bwd_gr, bwd_gi, proj_W, proj_b, prefix_emb, signal_emb):
    f64 = np.float64
    f16 = np.float16
    lam_f = np.exp(-np.exp(fwd_nu.astype(f64)) + 1j * fwd_theta.astype(f64))
    lam_b = np.exp(-np.exp(bwd_nu.astype(f64)) + 1j * bwd_theta.astype(f64))
    g_f = fwd_gr.astype(f64) + 1j * fwd_gi.astype(f64)
    g_b = bwd_gr.astype(f64) + 1j * bwd_gi.astype(f64)
    proj_Wd = proj_W.astype(f64)

    tau = np.arange(P)
    kf = np.real(g_f[None, :] * lam_f[None, :] ** tau[:, None])   # [64, D]
    kb = np.real(g_b[None, :] * lam_b[None, :] ** tau[:, None])
    Kf = np.fft.fft(kf, n=N, axis=0)                              # [128, D]
    Kb = np.fft.fft(kb, n=N, axis=0)
    Kfr, Kfi = np.real(Kf), np.imag(Kf)
    Kbr, Kbin = np.real(Kb), -np.imag(Kb)

    s = np.arange(N)
    f = np.arange(N)
    ang = 2 * np.pi * np.outer(s, f) / N
    FrT = np.cos(ang)                                             # [s, f]
    FiT = -np.sin(ang)
    t64 = np.arange(P)
    angi = 2 * np.pi * np.outer(f, t64) / N                       # [f, t]
    C = np.cos(angi) / N
    Sm = -np.sin(angi) / N

    jj = np.arange(T)
    lamj = lam_b[None, :] ** jj[:, None]                          # [T, D]
    Wr = np.real(lamj) / V
    Wi = np.imag(lamj) / V

    A = g_b[None, :] * lam_b[None, :] ** (P - t64)[:, None]       # [t, d]
    ArT = np.real(A).T                                            # [d, t]
    AinT = -np.imag(A).T

    cumkf = np.cumsum(kf, axis=0)
    cumkb = np.cumsum(kb, axis=0)
    pe = prefix_emb.reshape(-1).astype(f64)
    se = signal_emb.reshape(-1).astype(f64)
    y_pe_f = pe[None, :] * cumkf
    y_pe_b = pe[None, :] * cumkb[::-1, :]
    geo = np.sum(lam_b[None, :] ** np.arange(L_P)[:, None], axis=0)
    y_se_b = np.real(A * geo[None, :]) * se[None, :]
    Bfeat = np.concatenate([y_pe_f, y_pe_b + y_se_b], axis=1)     # [64, 2D]
    BT = (proj_b.astype(f64)[None, :] + Bfeat @ proj_Wd.T)        # [64, 256]

    # tblA tail [128, 256]: FrT | FiT  (joined to [x|mem] per batch)
    tblA_tail = np.concatenate([FrT, FiT], axis=1).astype(f16)
    # tblB: K1=[Kfr|Kfi] K2=[Kfi|Kfr] K3=[Kbr|-Kbi] K4=[-Kbi|Kbr] | C Sm |
    #       Ar_h0 Ar_h1 Ain_h0 Ain_h1 | W
    blocks = [Kfr, Kfi, Kfi, Kfr, Kbr, Kbin, Kbin, Kbr, C, Sm,
              ArT[:128], ArT[128:], AinT[:128], AinT[128:]]
    if T == 64:
        blocks.append(np.repeat(Wr, 2, axis=0))                   # [128, 256]
        blocks.append(np.repeat(Wi, 2, axis=0))
        n_wch = 1
    else:
        n_wch = T // 128
        for ch in range(n_wch):
            blocks.append(Wr[128 * ch:128 * (ch + 1)])
            blocks.append(Wi[128 * ch:128 * (ch + 1)])
    tblB = np.concatenate(blocks, axis=1).astype(f16)
    # tblC [128, 1280]: WP (4x[128,256]) | BT pad
    WpT = np.ascontiguousarray(proj_Wd.T)                         # [2D, 256]
    btpad = np.zeros((128, 256))
    btpad[:P] = BT
    tblC = np.concatenate([WpT[0:128], WpT[128:256], WpT[256:384],
                           WpT[384:512], btpad], axis=1).astype(f16)
    return {"tblA_tail": tblA_tail, "tblB": tblB, "tblC": tblC}


def _build_bass(T):
    import concourse.bacc as bacc
    import concourse.mybir as mybir
    from concourse.tile import TileContext

    dt = mybir.dt.float32
    dth = mybir.dt.float16
    nc = bacc.Bacc("TRN2", num_swdge_queues=1)

    n_ch = max(1, T // 128)
    WCOL = 2432                # K (2048) + C/Sm (128) + AT (256)
    if T == 64:
        tsx = nc.dram_tensor("tsx", (128, 1536), dth, kind="ExternalInput")
    else:
        tsx = nc.dram_tensor("tsx", (T, V * D), dth, kind="ExternalInput")
        tblAd = nc.dram_tensor("tblA", (N, 512), dth, kind="ExternalInput")
    tblBd = nc.dram_tensor("tblB", (N, WCOL + 512 * n_ch), dth,
                           kind="ExternalInput")
    tblCd = nc.dram_tensor("tblC", (N, 1280), dth, kind="ExternalInput")
    outd = nc.dram_tensor("out", (P, D), dt, kind="ExternalOutput")

    with TileContext(nc) as tc:
        with (
            tc.tile_pool(name="xin", bufs=min(n_ch, 2)) as xin_pool,
            tc.tile_pool(name="work", bufs=4) as work_pool,
            tc.tile_pool(name="const", bufs=1) as const_pool,
            tc.tile_pool(name="psz", bufs=1, space="PSUM") as psz_pool,
            tc.tile_pool(name="psf", bufs=1, space="PSUM") as psf_pool,
            tc.tile_pool(name="pst", bufs=1, space="PSUM") as pst_pool,
            tc.tile_pool(name="psp", bufs=1, space="PSUM") as psp_pool,
        ):
            # ---- input DMAs on three parallel rings ----
            tblB = const_pool.tile([N, WCOL + 512 * n_ch], dth)
            nc.scalar.dma_start(out=tblB[:], in_=tblBd[:])
            xs = []
            if T == 64:
                xa = xin_pool.tile([128, 1536], dth, tag="x")
                nc.sync.dma_start(out=xa[:], in_=tsx[:])
                xs.append(xa[:, 0:1024])
                tblA = xa[:, 1024:1536]
            else:
                tblA = const_pool.tile([N, 512], dth)
                nc.sync.dma_start(out=tblA[:], in_=tblAd[:])
                for ch in range(n_ch):
                    x = xin_pool.tile([128, 2048], dth, tag="x")
                    nc.sync.dma_start(out=x[:], in_=tsx[128 * ch:128 * ch + 128, :])
                    xs.append(x)
            tblC = const_pool.tile([N, 1280], dth)
            nc.gpsimd.dma_start(out=tblC[:], in_=tblCd[:])

            ones = const_pool.tile([128, 1], dth)
            nc.gpsimd.memset(ones[:], 1.0)

            mem_t = tblA[:, 0:256]
            FrT_t = tblA[:, 256:384]
            FiT_t = tblA[:, 384:512]
            Ct = tblB[:, 2048:2112]
            Smt = tblB[:, 2112:2176]
            ATt = tblB[:, 2176:2432]

            # ---- memory path: one DFT serves fwd+bwd ----
            zpsum = psz_pool.tile([N, 512], dt)
            nc.tensor.matmul(zpsum[:, 0:256], FrT_t, mem_t, start=True, stop=True)
            nc.tensor.matmul(zpsum[:, 256:512], FiT_t, mem_t, start=True, stop=True)
            zs = const_pool.tile([N, 512], dth)     # [zr|zi]
            nc.scalar.copy(out=zs[:], in_=zpsum[:])

            # ---- ts path: v-reduction + lam^j weighting ----
            st_psum = pst_pool.tile([128, 4], dt)
            if T == 64:
                x = xs[0]
                cc = work_pool.tile([128, 512], dth, tag="cc")
                nc.vector.tensor_add(out=cc[:], in0=x[:, 0:512], in1=x[:, 512:1024])
                a2 = work_pool.tile([128, 256], dth, tag="a2")
                nc.vector.tensor_add(out=a2[:], in0=cc[:, 0:256], in1=cc[:, 256:512])
                p = work_pool.tile([128, 512], dth, tag="p")
                nc.vector.tensor_mul(out=p[:, 0:256], in0=a2[:],
                                     in1=tblB[:, WCOL:WCOL + 256])
                nc.vector.tensor_mul(out=p[:, 256:512], in0=a2[:],
                                     in1=tblB[:, WCOL + 256:WCOL + 512])
                for g in range(4):
                    nc.tensor.matmul(st_psum[:, g:g + 1],
                                     p[:, 128 * g:128 * (g + 1)], ones[:],
                                     start=True, stop=True)
            else:
                for ch, x in enumerate(xs):
                    bb = work_pool.tile([128, 1024], dth, tag="bb")
                    nc.vector.tensor_add(out=bb[:], in0=x[:, 0:1024],
                                         in1=x[:, 1024:2048])
                    cc = work_pool.tile([128, 512], dth, tag="cc")
                    nc.vector.tensor_add(out=cc[:], in0=bb[:, 0:512],
                                         in1=bb[:, 512:1024])
                    a1 = work_pool.tile([128, 256], dth, tag="a2")
                    nc.vector.tensor_add(out=a1[:], in0=cc[:, 0:256],
                                         in1=cc[:, 256:512])
                    p = work_pool.tile([128, 512], dth, tag="p")
                    wof = WCOL + 512 * ch
                    nc.vector.tensor_mul(out=p[:, 0:256], in0=a1[:],
                                         in1=tblB[:, wof:wof + 256])
                    nc.vector.tensor_mul(out=p[:, 256:512], in0=a1[:],
                                         in1=tblB[:, wof + 256:wof + 512])
                    for g in range(4):
                        nc.tensor.matmul(st_psum[:, g:g + 1],
                                         p[:, 128 * g:128 * (g + 1)], ones[:],
                                         start=(ch == 0), stop=(ch == n_ch - 1))

            # ---- pointwise complex multiplies, 512-wide, all on [zr|zi] ----
            P1 = work_pool.tile([128, 512], dth, tag="P1")
            P2 = work_pool.tile([128, 512], dth, tag="P2")
            P3 = work_pool.tile([128, 512], dth, tag="P3")
            P4 = work_pool.tile([128, 512], dth, tag="P4")
            uf = const_pool.tile([128, 512], dth)    # [ufr|ufi]
            ub = const_pool.tile([128, 512], dth)    # [ubr|-ubi]
            nc.vector.tensor_mul(out=P1[:], in0=zs[:], in1=tblB[:, 0:512])
            nc.vector.tensor_mul(out=P2[:], in0=zs[:], in1=tblB[:, 512:1024])
            nc.vector.tensor_sub(out=uf[:, 0:256], in0=P1[:, 0:256], in1=P1[:, 256:512])
            nc.vector.tensor_add(out=uf[:, 256:512], in0=P2[:, 0:256], in1=P2[:, 256:512])
            nc.vector.tensor_mul(out=P3[:], in0=zs[:], in1=tblB[:, 1024:1536])
            nc.vector.tensor_mul(out=P4[:], in0=zs[:], in1=tblB[:, 1536:2048])
            nc.vector.tensor_sub(out=ub[:, 0:256], in0=P3[:, 0:256], in1=P3[:, 256:512])
            nc.vector.tensor_add(out=ub[:, 256:512], in0=P4[:, 0:256], in1=P4[:, 256:512])

            # ---- inverse transform into [d-block, t] feature blocks ----
            ffp = psf_pool.tile([128, 128], dt, tag="ffp")
            fbp = psf_pool.tile([128, 128], dt, tag="fbp")
            for h in range(2):
                nc.tensor.matmul(ffp[:, 64 * h:64 * h + 64],
                                 uf[:, 128 * h:128 * h + 128], Ct,
                                 start=True, stop=False)
                nc.tensor.matmul(ffp[:, 64 * h:64 * h + 64],
                                 uf[:, 256 + 128 * h:256 + 128 * h + 128], Smt,
                                 start=False, stop=True)
            for h in range(2):
                nc.tensor.matmul(fbp[:, 64 * h:64 * h + 64],
                                 ub[:, 128 * h:128 * h + 128], Ct,
                                 start=True, stop=False)
                nc.tensor.matmul(fbp[:, 64 * h:64 * h + 64],
                                 ub[:, 256 + 128 * h:256 + 128 * h + 128], Smt,
                                 start=False, stop=True)

            # ---- S-term (per-partition scales on ACT) + merge ----
            feat = const_pool.tile([128, 256], dth)
            uab = work_pool.tile([128, 128], dth, tag="uab")
            uaib = work_pool.tile([128, 128], dth, tag="uaib")
            for h in range(2):
                nc.vector.tensor_scalar_mul(uab[:, 64 * h:64 * h + 64],
                                            ATt[:, 64 * h:64 * h + 64],
                                            st_psum[:, h:h + 1])
                nc.vector.tensor_scalar_mul(uaib[:, 64 * h:64 * h + 64],
                                            ATt[:, 128 + 64 * h:192 + 64 * h],
                                            st_psum[:, 2 + h:3 + h])
            fb1 = work_pool.tile([128, 128], dth, tag="fb1")
            nc.vector.tensor_add(out=fb1[:], in0=fbp[:], in1=uab[:])
            nc.vector.tensor_add(out=feat[:, 128:256], in0=fb1[:], in1=uaib[:])
            nc.scalar.copy(out=feat[:, 0:128], in_=ffp[:])

            # ---- projection + bias + store ----
            pj = psp_pool.tile([P, D], dt)
            for g in range(4):
                nc.tensor.matmul(pj[:], feat[:, 64 * g:64 * (g + 1)],
                                 tblC[:, 256 * g:256 * (g + 1)],
                                 start=(g == 0), stop=(g == 3))
            out_sb = const_pool.tile([P, D], dt)
            nc.vector.tensor_add(out=out_sb[:], in0=pj[:], in1=tblC[0:P, 1024:1280])
            nc.sync.dma_start(out=outd[:], in_=out_sb[:])

    nc.compile()
    return nc


def _ensure_axon_hooks_shim():
    """bass_utils imports antenv.axon_hooks when tracing; some images lack it."""
    import sys, types
    try:
        import antenv  # noqa: F401
    except ImportError:
        return
    if "antenv.axon_hooks" in sys.modules:
        return
    try:
        from antenv import axon_hooks  # noqa: F401
        return
    except ImportError:
        pass
    hooks = types.ModuleType("antenv.axon_hooks")
    hooks._hook = None
    def _set(h):
        hooks._hook = h
    def _get():
        return hooks._hook
    hooks.set_axon_ntff_profile_hook = _set
    hooks.get_axon_ntff_profile_hook = _get
    sys.modules["antenv.axon_hooks"] = hooks


def kernel(**inputs):
    global LAST_RESULTS
    import os
    from concourse.bass_utils import run_bass_kernel_spmd
    _ensure_axon_hooks_shim()

    f16 = np.float16
    T = _pick_T(inputs["bwd_nu"])

    if T not in _CACHE:
        _CACHE[T] = _build_bass(T)
    nc = _CACHE[T]

    pkeys = ["fwd_nu", "fwd_theta", "fwd_gr", "fwd_gi", "bwd_nu", "bwd_theta",
             "bwd_gr", "bwd_gi", "proj_W", "proj_b", "prefix_emb", "signal_emb"]
    tables = _make_tables(T, **{k: np.asarray(inputs[k]) for k in pkeys})
    tblA_tail = tables.pop("tblA_tail")

    memory = np.asarray(inputs["memory"], np.float32)
    ts_embeds = np.asarray(inputs["ts_embeds"], np.float32)
    in_maps = []
    if T == 64:
        ts16 = ts_embeds[:, :T].reshape(B, 128, 1024).astype(f16)
        for b in range(B):
            memp = np.zeros((N, D), f16)
            memp[:P] = memory[b]
            tsxA = np.concatenate([ts16[b], memp, tblA_tail], axis=1)
            m = {"tsx": tsxA}
            m.update(tables)
            in_maps.append(m)
    else:
        ts16 = ts_embeds[:, :T].reshape(B, T, V * D).astype(f16)
        for b in range(B):
            memp = np.zeros((N, D), f16)
            memp[:P] = memory[b]
            tblA = np.concatenate([memp, tblA_tail], axis=1)
            m = {"tsx": ts16[b], "tblA": tblA}
            m.update(tables)
            in_maps.append(m)

    trace = os.environ.get("BASS_KERNEL_TRACE", "0") == "1"
    res = run_bass_kernel_spmd(nc, in_maps, core_ids=list(range(B)), trace=trace)
    LAST_RESULTS = res
    return np.stack([res.results[b]["out"] for b in range(B)], axis=0)


# revision 13
# speedup vs baseline: 1.0844x; 1.0499x over previous
"""Trainium2 Bass kernel for nn_MemoryTimeUnit.

Math: the reference keeps only Zp[:, :P] and averages over V. By linearity the
computation collapses to (per batch):
  out = proj(feat) + bias table, with
  y_fwd[t,d]  = causal 64-tap conv of memory with kf          (DFT-128 on device)
  y_bwd[t,d]  = anticausal conv with kb + Re{g_b lam_b^{P-t} S[d]}
  S[d] = sum_{j<T} lam_b^j * mean_v ts[b,j,v,d]
The backward DFT of the flipped memory equals conj(fwd DFT) with phases that
cancel exactly in the inverse transform, so one DFT (Zr,Zi) serves both paths.
Sign/swap variants of the K-hat tables are precomputed on host so all four
pointwise complex products read the same [zr|zi] operand (no swapped copy),
and fwd/bwd share the C/Sm inverse tables.

|lam_b| < 1 decays exponentially in j, so S is truncated adaptively at the
smallest T in {64, 128, 256, ...} whose predicted output error 0.04*max|lam|^T
is under 4e-3 (computed from bwd_nu at runtime; measured truncation error at
T=64 on randn-scale params is ~3e-3 against the 2e-2 gate).

Sharding: one batch per core (8 cores). Tables host-precomputed from the
per-channel params (O(D^2), data-independent) and replicated. Inputs
host-cast to fp16. For T=64 the ts slice is shipped as [128, 1024] (same
bytes) so the v-reduction uses all 128 DVE lanes; the lam^j weight table is
row-duplicated to match. Engine split: 2-src elementwise on DVE only (GpSimd
shares SBUF ports with DVE), casts + per-partition scales on ACT, matmuls on
PE; three DMA streams (sync/scalar HWDGE + gpsimd SWDGE), all contiguous.
"""

import numpy as np

B, P, V, L_P, D = 8, 64, 8, 1024, 256
N = 128

_CACHE = {}
LAST_RESULTS = None


def _pick_T(bwd_nu):
    max_abs_lam = float(np.exp(-np.exp(np.asarray(bwd_nu, np.float64))).max())
    for T in [64, 128, 256, 384, 512, 640, 768, 896, 1024]:
        if 0.04 * max_abs_lam ** T <= 4e-3 or T == 1024:
            return T
    return 1024


def _make_tables(T, fwd_nu, fwd_theta, fwd_gr, fwd_gi, bwd_nu, bwd_theta,
                 bwd_gr, bwd_gi, proj_W, proj_b, prefix_emb, signal_emb):
    f64 = np.float64
    f16 = np.float16
    lam_f = np.exp(-np.exp(fwd_nu.astype(f64)) + 1j * fwd_theta.astype(f64))
    lam_b = np.exp(-np.exp(bwd_nu.astype(f64)) + 1j * bwd_theta.astype(f64))
    g_f = fwd_gr.astype(f64) + 1j * fwd_gi.astype(f64)
    g_b = bwd_gr.astype(f64) + 1j * bwd_gi.astype(f64)
    proj_Wd = proj_W.astype(f64)

    tau = np.arange(P)
    kf = np.real(g_f[None, :] * lam_f[None, :] ** tau[:, None])   # [64, D]
    kb = np.real(g_b[None, :] * lam_b[None, :] ** tau[:, None])
    Kf = np.fft.fft(kf, n=N, axis=0)                              # [128, D]
    Kb = np.fft.fft(kb, n=N, axis=0)
    Kfr, Kfi = np.real(Kf), np.imag(Kf)
    Kbr, Kbin = np.real(Kb), -np.imag(Kb)

    s = np.arange(N)
    f = np.arange(N)
    ang = 2 * np.pi * np.outer(s, f) / N
    FrT = np.cos(ang)                                             # [s, f]
    FiT = -np.sin(ang)
    t64 = np.arange(P)
    angi = 2 * np.pi * np.outer(f, t64) / N                       # [f, t]
    C = np.cos(angi) / N
    Sm = -np.sin(angi) / N

    jj = np.arange(T)
    lamj = lam_b[None, :] ** jj[:, None]                          # [T, D]
    Wr = np.real(lamj) / V
    Wi = np.imag(lamj) / V

    A = g_b[None, :] * lam_b[None, :] ** (P - t64)[:, None]       # [t, d]
    ArT = np.real(A).T                                            # [d, t]
    AinT = -np.imag(A).T

    cumkf = np.cumsum(kf, axis=0)
    cumkb = np.cumsum(kb, axis=0)
    pe = prefix_emb.reshape(-1).astype(f64)
    se = signal_emb.reshape(-1).astype(f64)
    y_pe_f = pe[None, :] * cumkf
    y_pe_b = pe[None, :] * cumkb[::-1, :]
    geo = np.sum(lam_b[None, :] ** np.arange(L_P)[:, None], axis=0)
    y_se_b = np.real(A * geo[None, :]) * se[None, :]
    Bfeat = np.concatenate([y_pe_f, y_pe_b + y_se_b], axis=1)     # [64, 2D]
    BT = (proj_b.astype(f64)[None, :] + Bfeat @ proj_Wd.T)        # [64, 256]

    # tblA tail [128, 256]: FrT | FiT  (joined to [x|mem] per batch)
    tblA_tail = np.concatenate([FrT, FiT], axis=1).astype(f16)
    # tblB: K1=[Kfr|Kfi] K2=[Kfi|Kfr] K3=[Kbr|-Kbi] K4=[-Kbi|Kbr] | C Sm |
    #       Ar_h0 Ar_h1 Ain_h0 Ain_h1 | W
    blocks = [Kfr, Kfi, Kfi, Kfr, Kbr, Kbin, Kbin, Kbr, C, Sm,
              ArT[:128], ArT[128:], AinT[:128], AinT[128:]]
    if T == 64:
        blocks.append(np.repeat(Wr, 2, axis=0))                   # [128, 256]
        blocks.append(np.repeat(Wi, 2, axis=0))
        n_wch = 1
    else:
        n_wch = T // 128
        for ch in range(n_wch):
            blocks.append(Wr[128 * ch:128 * (ch + 1)])
            blocks.append(Wi[128 * ch:128 * (ch + 1)])
    tblB = np.concatenate(blocks, axis=1).astype(f16)
    # tblC [128, 1280]: WP (4x[128,256]) | BT pad
    WpT = np.ascontiguousarray(proj_Wd.T)                         # [2D, 256]
    btpad = np.zeros((128, 256))
    btpad[:P] = BT
    tblC = np.concatenate([WpT[0:128], WpT[128:256], WpT[256:384],
                           WpT[384:512], btpad], axis=1).astype(f16)
    return {"tblA_tail": tblA_tail, "tblB": tblB, "tblC": tblC}


def _patch_tile_epilogue():
    """Drop the trailing all-engine butterfly after Tile's semaphore cleanup.

    Tile's exit emits drain + barrier + sem-clears + barrier. The final
    barrier only re-synchronizes already-idle engines (costs ~2-4us of serial
    cross-engine semaphore hops at the very end of the NEFF); the preceding
    barrier already fences the clears against live work, and the runtime
    waits for every engine to halt regardless.
    """
    from concourse.tile import TileContext
    from concourse.vector_clock import ScopedClock
    if getattr(TileContext, "_cheap_epilogue", False):
        return
    def _drain_and_barrier(self, tick_clock, wait_clock):
        drain_inst = self.nc.sync.drain()
        wait_clock.add_sem_waits(
            drain_inst.ins, ScopedClock({None: tick_clock.global_clock})
        )
        self.nc.all_engine_barrier()
        assert self.sems is not None
        popped = self.nc._tile_sem_poison_stack.pop()
        assert popped is self._sem_poison
        self.nc.clear_and_free_semaphores(list(self.sems.allocated().values()))
    TileContext._drain_and_barrier = _drain_and_barrier
    TileContext._cheap_epilogue = True


def _build_bass(T):
    import concourse.bacc as bacc
    import concourse.mybir as mybir
    from concourse.tile import TileContext
    _patch_tile_epilogue()

    dt = mybir.dt.float32
    dth = mybir.dt.float16
    nc = bacc.Bacc("TRN2", num_swdge_queues=1)

    n_ch = max(1, T // 128)
    WCOL = 2432                # K (2048) + C/Sm (128) + AT (256)
    if T == 64:
        tsx = nc.dram_tensor("tsx", (128, 1536), dth, kind="ExternalInput")
    else:
        tsx = nc.dram_tensor("tsx", (T, V * D), dth, kind="ExternalInput")
        tblAd = nc.dram_tensor("tblA", (N, 512), dth, kind="ExternalInput")
    tblBd = nc.dram_tensor("tblB", (N, WCOL + 512 * n_ch), dth,
                           kind="ExternalInput")
    tblCd = nc.dram_tensor("tblC", (N, 1280), dth, kind="ExternalInput")
    outd = nc.dram_tensor("out", (P, D), dt, kind="ExternalOutput")

    with TileContext(nc) as tc:
        with (
            tc.tile_pool(name="xin", bufs=min(n_ch, 2)) as xin_pool,
            tc.tile_pool(name="work", bufs=4) as work_pool,
            tc.tile_pool(name="const", bufs=1) as const_pool,
            tc.tile_pool(name="psz", bufs=1, space="PSUM") as psz_pool,
            tc.tile_pool(name="psf", bufs=1, space="PSUM") as psf_pool,
            tc.tile_pool(name="pst", bufs=1, space="PSUM") as pst_pool,
            tc.tile_pool(name="psp", bufs=1, space="PSUM") as psp_pool,
        ):
            # ---- input DMAs on three parallel rings ----
            tblB = const_pool.tile([N, WCOL + 512 * n_ch], dth)
            nc.scalar.dma_start(out=tblB[:], in_=tblBd[:])
            xs = []
            if T == 64:
                xa = xin_pool.tile([128, 1536], dth, tag="x")
                nc.sync.dma_start(out=xa[:], in_=tsx[:])
                xs.append(xa[:, 0:1024])
                tblA = xa[:, 1024:1536]
            else:
                tblA = const_pool.tile([N, 512], dth)
                nc.sync.dma_start(out=tblA[:], in_=tblAd[:])
                for ch in range(n_ch):
                    x = xin_pool.tile([128, 2048], dth, tag="x")
                    nc.sync.dma_start(out=x[:], in_=tsx[128 * ch:128 * ch + 128, :])
                    xs.append(x)
            tblC = const_pool.tile([N, 1280], dth)
            nc.gpsimd.dma_start(out=tblC[:], in_=tblCd[:])

            ones = const_pool.tile([128, 1], dth)
            nc.gpsimd.memset(ones[:], 1.0)

            mem_t = tblA[:, 0:256]
            FrT_t = tblA[:, 256:384]
            FiT_t = tblA[:, 384:512]
            Ct = tblB[:, 2048:2112]
            Smt = tblB[:, 2112:2176]
            ATt = tblB[:, 2176:2432]

            # ---- memory path: one DFT serves fwd+bwd ----
            zpsum = psz_pool.tile([N, 512], dt)
            nc.tensor.matmul(zpsum[:, 0:256], FrT_t, mem_t, start=True, stop=True)
            nc.tensor.matmul(zpsum[:, 256:512], FiT_t, mem_t, start=True, stop=True)
            zs = const_pool.tile([N, 512], dth)     # [zr|zi]
            nc.scalar.copy(out=zs[:], in_=zpsum[:])

            # ---- ts path: v-reduction + lam^j weighting ----
            st_psum = pst_pool.tile([128, 4], dt)
            if T == 64:
                x = xs[0]
                cc = work_pool.tile([128, 512], dth, tag="cc")
                nc.vector.tensor_add(out=cc[:], in0=x[:, 0:512], in1=x[:, 512:1024])
                a2 = work_pool.tile([128, 256], dth, tag="a2")
                nc.vector.tensor_add(out=a2[:], in0=cc[:, 0:256], in1=cc[:, 256:512])
                p = work_pool.tile([128, 512], dth, tag="p")
                nc.vector.tensor_mul(out=p[:, 0:256], in0=a2[:],
                                     in1=tblB[:, WCOL:WCOL + 256])
                nc.vector.tensor_mul(out=p[:, 256:512], in0=a2[:],
                                     in1=tblB[:, WCOL + 256:WCOL + 512])
                for g in range(4):
                    nc.tensor.matmul(st_psum[:, g:g + 1],
                                     p[:, 128 * g:128 * (g + 1)], ones[:],
                                     start=True, stop=True)
            else:
                for ch, x in enumerate(xs):
                    bb = work_pool.tile([128, 1024], dth, tag="bb")
                    nc.vector.tensor_add(out=bb[:], in0=x[:, 0:1024],
                                         in1=x[:, 1024:2048])
                    cc = work_pool.tile([128, 512], dth, tag="cc")
                    nc.vector.tensor_add(out=cc[:], in0=bb[:, 0:512],
                                         in1=bb[:, 512:1024])
                    a1 = work_pool.tile([128, 256], dth, tag="a2")
                    nc.vector.tensor_add(out=a1[:], in0=cc[:, 0:256],
                                         in1=cc[:, 256:512])
                    p = work_pool.tile([128, 512], dth, tag="p")
                    wof = WCOL + 512 * ch
                    nc.vector.tensor_mul(out=p[:, 0:256], in0=a1[:],
                                         in1=tblB[:, wof:wof + 256])
                    nc.vector.tensor_mul(out=p[:, 256:512], in0=a1[:],
                                         in1=tblB[:, wof + 256:wof + 512])
                    for g in range(4):
                        nc.tensor.matmul(st_psum[:, g:g + 1],
                                         p[:, 128 * g:128 * (g + 1)], ones[:],
                                         start=(ch == 0), stop=(ch == n_ch - 1))

            # ---- pointwise complex multiplies, 512-wide, all on [zr|zi] ----
            P1 = work_pool.tile([128, 512], dth, tag="P1")
            P2 = work_pool.tile([128, 512], dth, tag="P2")
            P3 = work_pool.tile([128, 512], dth, tag="P3")
            P4 = work_pool.tile([128, 512], dth, tag="P4")
            uf = const_pool.tile([128, 512], dth)    # [ufr|ufi]
            ub = const_pool.tile([128, 512], dth)    # [ubr|-ubi]
            nc.vector.tensor_mul(out=P1[:], in0=zs[:], in1=tblB[:, 0:512])
            nc.vector.tensor_mul(out=P2[:], in0=zs[:], in1=tblB[:, 512:1024])
            nc.vector.tensor_sub(out=uf[:, 0:256], in0=P1[:, 0:256], in1=P1[:, 256:512])
            nc.vector.tensor_add(out=uf[:, 256:512], in0=P2[:, 0:256], in1=P2[:, 256:512])
            nc.vector.tensor_mul(out=P3[:], in0=zs[:], in1=tblB[:, 1024:1536])
            nc.vector.tensor_mul(out=P4[:], in0=zs[:], in1=tblB[:, 1536:2048])
            nc.vector.tensor_sub(out=ub[:, 0:256], in0=P3[:, 0:256], in1=P3[:, 256:512])
            nc.vector.tensor_add(out=ub[:, 256:512], in0=P4[:, 0:256], in1=P4[:, 256:512])

            # ---- inverse transform into [d-block, t] feature blocks ----
            ffp = psf_pool.tile([128, 128], dt, tag="ffp")
            fbp = psf_pool.tile([128, 128], dt, tag="fbp")
            for h in range(2):
                nc.tensor.matmul(ffp[:, 64 * h:64 * h + 64],
                                 uf[:, 128 * h:128 * h + 128], Ct,
                                 start=True, stop=False)
                nc.tensor.matmul(ffp[:, 64 * h:64 * h + 64],
                                 uf[:, 256 + 128 * h:256 + 128 * h + 128], Smt,
                                 start=False, stop=True)
            for h in range(2):
                nc.tensor.matmul(fbp[:, 64 * h:64 * h + 64],
                                 ub[:, 128 * h:128 * h + 128], Ct,
                                 start=True, stop=False)
                nc.tensor.matmul(fbp[:, 64 * h:64 * h + 64],
                                 ub[:, 256 + 128 * h:256 + 128 * h + 128], Smt,
                                 start=False, stop=True)

            # ---- S-term (per-partition scales on ACT) + merge ----
            feat = const_pool.tile([128, 256], dth)
            uab = work_pool.tile([128, 128], dth, tag="uab")
            uaib = work_pool.tile([128, 128], dth, tag="uaib")
            for h in range(2):
                nc.vector.tensor_scalar_mul(uab[:, 64 * h:64 * h + 64],
                                            ATt[:, 64 * h:64 * h + 64],
                                            st_psum[:, h:h + 1])
                nc.vector.tensor_scalar_mul(uaib[:, 64 * h:64 * h + 64],
                                            ATt[:, 128 + 64 * h:192 + 64 * h],
                                            st_psum[:, 2 + h:3 + h])
            fb1 = work_pool.tile([128, 128], dth, tag="fb1")
            nc.vector.tensor_add(out=fb1[:], in0=fbp[:], in1=uab[:])
            nc.vector.tensor_add(out=feat[:, 128:256], in0=fb1[:], in1=uaib[:])
            nc.scalar.copy(out=feat[:, 0:128], in_=ffp[:])

            # ---- projection + bias + store ----
            pj = psp_pool.tile([P, D], dt)
            for g in range(4):
                nc.tensor.matmul(pj[:], feat[:, 64 * g:64 * (g + 1)],
                                 tblC[:, 256 * g:256 * (g + 1)],
                                 start=(g == 0), stop=(g == 3))
            out_sb = const_pool.tile([P, D], dt)
            nc.vector.tensor_add(out=out_sb[:], in0=pj[:], in1=tblC[0:P, 1024:1280])
            nc.sync.dma_start(out=outd[:], in_=out_sb[:])

    nc.compile()
    return nc


def _ensure_axon_hooks_shim():
    """bass_utils imports antenv.axon_hooks when tracing; some images lack it."""
    import sys, types
    try:
        import antenv  # noqa: F401
    except ImportError:
        return
    if "antenv.axon_hooks" in sys.modules:
        return
    try:
        from antenv import axon_hooks  # noqa: F401
        return
    except ImportError:
        pass
    hooks = types.ModuleType("antenv.axon_hooks")
    hooks._hook = None
    def _set(h):
        hooks._hook = h
    def _get():
        return hooks._hook
    hooks.set_axon_ntff_profile_hook = _set
    hooks.get_axon_ntff_profile_hook = _get
    sys.modules["antenv.axon_hooks"] = hooks


def kernel(**inputs):
    global LAST_RESULTS
    import os
    from concourse.bass_utils import run_bass_kernel_spmd
    _ensure_axon_hooks_shim()

    f16 = np.float16
    T = _pick_T(inputs["bwd_nu"])

    if T not in _CACHE:
        _CACHE[T] = _build_bass(T)
    nc = _CACHE[T]

    pkeys = ["fwd_nu", "fwd_theta", "fwd_gr", "fwd_gi", "bwd_nu", "bwd_theta",
             "bwd_gr", "bwd_gi", "proj_W", "proj_b", "prefix_emb", "signal_emb"]
    tables = _make_tables(T, **{k: np.asarray(inputs[k]) for k in pkeys})
    tblA_tail = tables.pop("tblA_tail")

    memory = np.asarray(inputs["memory"], np.float32)
    ts_embeds = np.asarray(inputs["ts_embeds"], np.float32)
    in_maps = []
    if T == 64:
        ts16 = ts_embeds[:, :T].reshape(B, 128, 1024).astype(f16)
        for b in range(B):
            memp = np.zeros((N, D), f16)
            memp[:P] = memory[b]
            tsxA = np.concatenate([ts16[b], memp, tblA_tail], axis=1)
            m = {"tsx": tsxA}
            m.update(tables)
            in_maps.append(m)
    else:
        ts16 = ts_embeds[:, :T].reshape(B, T, V * D).astype(f16)
        for b in range(B):
            memp = np.zeros((N, D), f16)
            memp[:P] = memory[b]
            tblA = np.concatenate([memp, tblA_tail], axis=1)
            m = {"tsx": ts16[b], "tblA": tblA}
            m.update(tables)
            in_maps.append(m)

    trace = os.environ.get("BASS_KERNEL_TRACE", "0") == "1"
    res = run_bass_kernel_spmd(nc, in_maps, core_ids=list(range(B)), trace=trace)
    LAST_RESULTS = res
    return np.stack([res.results[b]["out"] for b in range(B)], axis=0)
